# revision 3
# baseline (speedup 1.0000x reference)
"""Trainium2 Bass kernel for LogicGatedSNN.

Math:
  w = ternarize(synapse_states)            # {-1,0,1}, exact in bf16
  current = spike_input @ w.T              # bf16 matmul, fp32 PSUM accum -> exact
  spikes[b,o] = (current[b,o] - T[o] >= 0)
where T[o] folds threshold/membrane/refractory:
  non-refractory: T = thr - DECAY*vmem
  refractory:     T = +-1e30 depending on sign of (DECAY*vmem - thr)
The -T[o] bias is added as one extra K=1 fp32 matmul into the same PSUM
accumulation group, so the epilogue is a single tensor_scalar is_ge.

Sharding: 8 cores = 2 (batch) x 4 (out_features). Per core:
  spike shard [2048, 4096], synapse shard [1024, 4096].
On-chip dataflow per core:
  - synapse: fp32 DMA in -> DVE ternarize (is_gt / is_lt / sub) -> bf16
    -> xbar DMA-transpose into W[128, 32, OS] (W[p, m, o] = w[o, 128m+p])
  - spike: SWDGE cast-DMA (fp32->bf16) -> xbar transpose S[128, 32, 128]
  - matmul: psum[128b, 512o] accumulates 32 chunks (K=128 each) + bias mm
  - DVE is_ge vs 0 -> fp32 out tile -> DMA out
"""

import sys

if "/opt/trn_rl_repo" not in sys.path:
    sys.path.insert(0, "/opt/trn_rl_repo")

import numpy as np

B, IN, OUT = 4096, 4096, 4096
GB, GO = 2, 4  # core grid: batch x out_features
DECAY = 0.8


def build_core_program(nc, tc, bs, os_, in_):
    """Emit the per-core program. bs/os_/in_ = per-core shard dims."""
    import concourse.mybir as mybir
    from concourse.bass import ts

    FP32 = mybir.dt.float32
    BF16 = mybir.dt.bfloat16
    Op = mybir.AluOpType

    spike = nc.dram_tensor("spike", [bs, in_], FP32, kind="ExternalInput")
    syn = nc.dram_tensor("syn", [os_, in_], FP32, kind="ExternalInput")
    thr = nc.dram_tensor("thr", [1, os_], FP32, kind="ExternalInput")
    vmem = nc.dram_tensor("vmem", [1, os_], FP32, kind="ExternalInput")
    refrac = nc.dram_tensor("refrac", [1, os_], FP32, kind="ExternalInput")
    out = nc.dram_tensor("spikes", [bs, os_], FP32, kind="ExternalOutput")

    KC = in_ // 128  # contraction chunks
    NB = bs // 128  # batch tiles
    NT = 512  # matmul free dim per o-tile
    NO = os_ // NT  # o-tiles

    with (
        tc.tile_pool(name="wpool", bufs=1) as wpool,
        tc.tile_pool(name="synpool", bufs=2) as synpool,
        tc.tile_pool(name="ternpool", bufs=2) as ternpool,
        tc.tile_pool(name="sppool", bufs=2) as sppool,
        tc.tile_pool(name="spool", bufs=3) as spool,
        tc.tile_pool(name="outpool", bufs=4) as outpool,
        tc.tile_pool(name="miscpool", bufs=1) as miscpool,
        tc.tile_pool(name="pspool", bufs=4, space="PSUM") as pspool,
    ):
        # ---- threshold vector negT[0, o] = -(effective threshold) ----
        tv = miscpool.tile([1, os_], FP32, tag="tv")
        vv = miscpool.tile([1, os_], FP32, tag="vv")
        rv = miscpool.tile([1, os_], FP32, tag="rv")
        nc.sync.dma_start(tv[:], thr[:, :])
        nc.sync.dma_start(vv[:], vmem[:, :])
        nc.sync.dma_start(rv[:], refrac[:, :])
        c0 = miscpool.tile([1, os_], FP32, tag="c0")
        nc.vector.tensor_scalar(c0[:], vv[:], DECAY, None, Op.mult)
        nc.vector.tensor_tensor(c0[:], c0[:], tv[:], Op.subtract)  # decay*v - thr
        big = miscpool.tile([1, os_], FP32, tag="big")
        nc.vector.tensor_scalar(big[:], c0[:], 0.0, None, Op.is_ge)
        nc.vector.tensor_scalar(big[:], big[:], 2e30, -1e30, Op.mult, Op.add)
        r01 = miscpool.tile([1, os_], FP32, tag="r01")
        nc.vector.tensor_scalar(r01[:], rv[:], 0.0, None, Op.is_gt)
        # negT = c0 + r01 * (big - c0)
        nc.vector.tensor_tensor(big[:], big[:], c0[:], Op.subtract)
        nc.vector.tensor_tensor(big[:], big[:], r01[:], Op.mult)
        negT = miscpool.tile([1, os_], FP32, tag="negT")
        nc.vector.tensor_tensor(negT[:], c0[:], big[:], Op.add)
        ones = miscpool.tile([1, 128], FP32, tag="ones")
        nc.vector.memset(ones[:], 1.0)

        # ---- weights: ternarize + transpose into W[p, m, o] = w[o, 128m+p] ----
        W = wpool.tile([128, KC, os_], BF16, tag="W")
        for j in range(os_ // 128):
            st = synpool.tile([128, in_], FP32, tag="st")
            nc.sync.dma_start(st[:], syn[ts(j, 128), :])
            ta = ternpool.tile([128, in_], BF16, tag="ta")
            tb = ternpool.tile([128, in_], BF16, tag="tb")
            nc.vector.tensor_scalar(ta[:], st[:], 1.0, None, Op.is_gt)
            nc.vector.tensor_scalar(tb[:], st[:], -1.0, None, Op.is_lt)
            nc.vector.tensor_tensor(ta[:], ta[:], tb[:], Op.subtract)
            nc.sync.dma_start(W[:, :, ts(j, 128)], ta[:], transpose=True)

        # ---- main sweep over batch tiles ----
        for bt in range(NB):
            sp = sppool.tile([128, in_], BF16, tag="sp")
            nc.gpsimd.dma_start(sp[:], spike[ts(bt, 128), :])  # fp32->bf16 cast
            S = spool.tile([128, KC, 128], BF16, tag="S")
            nc.sync.dma_start(S[:], sp[:], transpose=True)
            for ot in range(NO):
                ps = pspool.tile([128, NT], FP32, tag="ps")
                for m in range(KC):
                    nc.tensor.matmul(
                        ps[:],
                        S[:, m, :],
                        W[:, m, ts(ot, NT)],
                        start=(m == 0),
                        stop=False,
                    )
                nc.tensor.matmul(
                    ps[:], ones[:], negT[:, ts(ot, NT)], start=False, stop=True
                )
                ob = outpool.tile([128, NT], FP32, tag="ob")
                nc.vector.tensor_scalar(ob[:], ps[:], 0.0, None, Op.is_ge)
                nc.sync.dma_start(out[ts(bt, 128), ts(ot, NT)], ob[:])
    return out


def make_nc(bs=B // GB, os_=OUT // GO, in_=IN):
    from concourse import bacc
    from concourse.tile import TileContext

    nc = bacc.Bacc(trn_type="TRN2")
    with TileContext(nc) as tc:
        build_core_program(nc, tc, bs, os_, in_)
    nc.compile()
    return nc


_NC_CACHE = {}


def kernel(
    spike_input,
    synapse_states,
    membrane_potential,
    adaptive_threshold,
    refractory_count,
    _return_results=False,
):
    from concourse.bass_utils import run_bass_kernel_spmd

    spike_input = np.ascontiguousarray(np.asarray(spike_input, dtype=np.float32))
    synapse_states = np.ascontiguousarray(np.asarray(synapse_states, dtype=np.float32))
    membrane_potential = np.asarray(membrane_potential, dtype=np.float32)
    adaptive_threshold = np.asarray(adaptive_threshold, dtype=np.float32)
    refractory_count = np.asarray(refractory_count, dtype=np.float32)

    bs, os_ = B // GB, OUT // GO
    if "nc" not in _NC_CACHE:
        _NC_CACHE["nc"] = make_nc(bs, os_, IN)
    nc = _NC_CACHE["nc"]

    in_maps = []
    for c in range(GB * GO):
        bi, oj = divmod(c, GO)
        in_maps.append(
            {
                "spike": spike_input[bi * bs : (bi + 1) * bs],
                "syn": np.ascontiguousarray(
                    synapse_states[oj * os_ : (oj + 1) * os_]
                ),
                "thr": adaptive_threshold[None, oj * os_ : (oj + 1) * os_],
                "vmem": membrane_potential[None, oj * os_ : (oj + 1) * os_],
                "refrac": refractory_count[None, oj * os_ : (oj + 1) * os_],
            }
        )

    res = run_bass_kernel_spmd(nc, in_maps, core_ids=list(range(GB * GO)))

    full = np.empty((B, OUT), dtype=np.float32)
    for c in range(GB * GO):
        bi, oj = divmod(c, GO)
        full[bi * bs : (bi + 1) * bs, oj * os_ : (oj + 1) * os_] = res.results[c][
            "spikes"
        ]
    if _return_results:
        return full, res
    return full
